# revision 21
# baseline (speedup 1.0000x reference)
"""Embedding-lookup + row-wise dot kernel for Trainium2 (8 NeuronCores).

v7: 2x2 pair-window InstDMAGatherAnt (512B elements), raw Block mode.
Tables viewed as row-PAIRS double the int16 window to 65536 rows: user
shard (131072 rows, vocab-sharded 8 ways, host all-to-all) and movie
table (replicated) are 2 windows each -> 2x2 cells = 8 gather
instructions/core on 4 SWDGE queues (vs 32 in the 4x4 exact design;
~45us measured there).  Row-of-pair selection on DVE via expanded
[P,C,64] masks with FULL-WIDTH ops only (u_sel = lo + au*(hi-lo)) -
[P,4,C] dim-1-sliced APs compute garbage on HW (measured; avoid).
Gather geometry + select both verified exact single-core.
"""

import os
import numpy as np

N_USERS = 1_000_000
N_MOVIES = 100_000
EMB = 64
BATCH = 16384
N_CORES = 8
P = 128
SHARD_ROWS = 131072
WP = 32768
N_UW = 2
N_MW = 2
U_PAIRS = SHARD_ROWS // 2
M_PAIRS = N_MOVIES // 2

_CACHE = {}


def _ceil(a, b):
    return -(-a // b)


def _plan(users, movies):
    core = np.minimum(users // SHARD_ROWS, N_CORES - 1)
    plans = []
    counts = np.zeros((N_CORES, N_UW, N_MW), dtype=np.int64)
    for c in range(N_CORES):
        sel = np.flatnonzero(core == c)
        ulocal = users[sel] - c * SHARD_ROWS
        uwl = (ulocal >> 1) >> 15
        mwl = (movies[sel] >> 1) >> 15
        order = np.lexsort((mwl, uwl))
        sel = sel[order]
        uwl = uwl[order]
        mwl = mwl[order]
        for i in range(N_UW):
            for j in range(N_MW):
                counts[c, i, j] = int(np.sum((uwl == i) & (mwl == j)))
        plans.append({"elems": sel, "uwl": uwl, "mwl": mwl})
    cnt_max = counts.max(axis=0)
    assert int(cnt_max.max()) <= 1024
    cap = _ceil(np.maximum(cnt_max, 1), 128) * 128
    return plans, counts, cnt_max, cap


def _build_nc(cap, cnt_max):
    import concourse.bacc as bacc
    from concourse import mybir
    from concourse.library_config import mlp

    ncols = cap // 128
    C = int(ncols.sum())
    L16 = [[max(16, _ceil(int(cnt_max[i, j]), 16) * 16) for j in range(N_MW)] for i in range(N_UW)]
    idx_cols = sum(sum(r) for r in L16) // 16

    nc = bacc.Bacc(None, target_bir_lowering=False, num_swdge_queues=4)
    ushard_t = nc.dram_tensor("user_shard", [U_PAIRS, 2 * EMB], mybir.dt.float32, kind="ExternalInput")
    mtable_t = nc.dram_tensor("movie_table", [M_PAIRS, 2 * EMB], mybir.dt.float32, kind="ExternalInput")
    uidx_t = nc.dram_tensor("u_idx", [P, idx_cols], mybir.dt.int16, kind="ExternalInput")
    midx_t = nc.dram_tensor("m_idx", [P, idx_cols], mybir.dt.int16, kind="ExternalInput")
    au_t = nc.dram_tensor("au64", [P, C, EMB], mybir.dt.float32, kind="ExternalInput")
    am_t = nc.dram_tensor("am64", [P, C, EMB], mybir.dt.float32, kind="ExternalInput")
    out_t = nc.dram_tensor("out", [P, C], mybir.dt.float32, kind="ExternalOutput")

    with (
        nc.Block() as block,
        nc.sbuf_tensor("uidx", [P, idx_cols], mybir.dt.int16) as uidx,
        nc.sbuf_tensor("midx", [P, idx_cols], mybir.dt.int16) as midx,
        nc.sbuf_tensor("au_sb", [P, C, EMB], mybir.dt.float32) as au64,
        nc.sbuf_tensor("am_sb", [P, C, EMB], mybir.dt.float32) as am64,
        nc.sbuf_tensor("usel", [P, C, EMB], mybir.dt.float32) as usel,
        nc.sbuf_tensor("msel", [P, C, EMB], mybir.dt.float32) as msel,
        nc.sbuf_tensor("U2", [P, C, 2 * EMB], mybir.dt.float32) as U2,
        nc.sbuf_tensor("M2", [P, C, 2 * EMB], mybir.dt.float32) as M2,
        nc.sbuf_tensor("prod", [P, C, EMB], mybir.dt.float32) as prod,
        nc.sbuf_tensor("res", [P, C], mybir.dt.float32) as res,
        nc.semaphore("idx_sem") as idx_sem,
        nc.semaphore("gat0") as gat0,
        nc.semaphore("gat1") as gat1,
        nc.semaphore("cmp_sem") as cmp_sem,
        nc.semaphore("out_sem") as out_sem,
    ):
        gat_sems = [gat0, gat1]

        colbase = {}
        cb = 0
        off16 = {}
        o = 0
        for i in range(N_UW):
            for j in range(N_MW):
                colbase[(i, j)] = cb
                cb += int(ncols[i, j])
                off16[(i, j)] = o
                o += L16[i][j] // 16

        sched = []
        rr = 0
        for i in range(N_UW):
            for j in range(N_MW):
                sched.append(("u", i, j, rr % 4))
                rr += 1
                sched.append(("m", i, j, rr % 4))
                rr += 1

        @block.sync
        def _(sync):
            sync.dma_start(out=uidx[:], in_=uidx_t[:]).then_inc(idx_sem, 16)
            sync.dma_start(out=midx[:], in_=midx_t[:]).then_inc(idx_sem, 16)
            sync.dma_start(out=au64[:], in_=au_t[:]).then_inc(idx_sem, 16)
            sync.dma_start(out=am64[:], in_=am_t[:]).then_inc(idx_sem, 16)
            sync.wait_ge(cmp_sem, N_UW)
            sync.dma_start(out=out_t[:], in_=res[:]).then_inc(out_sem, 16)
            sync.wait_ge(out_sem, 16)

        @block.gpsimd
        def _(gpsimd):
            gpsimd.load_library(mlp)
            gpsimd.wait_ge(idx_sem, 32)
            for kind, i, j, q in sched:
                n = max(16, int(cnt_max[i, j]))
                dst_col = colbase[(i, j)]
                ncol_ij = int(ncols[i, j])
                o16 = off16[(i, j)]
                nl16 = L16[i][j] // 16
                if kind == "u":
                    gpsimd.dma_gather(
                        out_ap=U2[:, dst_col : dst_col + ncol_ij],
                        in_ap=ushard_t[i * WP : (i + 1) * WP],
                        idxs_ap=uidx[:, o16 : o16 + nl16],
                        num_idxs=n,
                        num_idxs_reg=n,
                        elem_size=2 * EMB,
                        queue_num=q,
                    ).then_inc(gat_sems[i], 16)
                else:
                    ext = min(WP, M_PAIRS - j * WP)
                    gpsimd.dma_gather(
                        out_ap=M2[:, dst_col : dst_col + ncol_ij],
                        in_ap=mtable_t[j * WP : j * WP + ext],
                        idxs_ap=midx[:, o16 : o16 + nl16],
                        num_idxs=n,
                        num_idxs_reg=n,
                        elem_size=2 * EMB,
                        queue_num=q,
                    ).then_inc(gat_sems[i], 16)

        @block.vector
        def _(vector):
            vector.wait_ge(idx_sem, 64)
            for i in range(N_UW):
                c0 = colbase[(i, 0)]
                nc_i = sum(int(ncols[i, j]) for j in range(N_MW))
                s_ = slice(c0, c0 + nc_i)
                vector.wait_ge(gat_sems[i], 16 * 2 * N_MW)
                vector.tensor_sub(out=usel[:, s_], in0=U2[:, s_, EMB : 2 * EMB], in1=U2[:, s_, 0:EMB])
                vector.tensor_mul(out=usel[:, s_], in0=usel[:, s_], in1=au64[:, s_])
                vector.tensor_add(out=usel[:, s_], in0=usel[:, s_], in1=U2[:, s_, 0:EMB])
                vector.tensor_sub(out=msel[:, s_], in0=M2[:, s_, EMB : 2 * EMB], in1=M2[:, s_, 0:EMB])
                vector.tensor_mul(out=msel[:, s_], in0=msel[:, s_], in1=am64[:, s_])
                vector.tensor_add(out=msel[:, s_], in0=msel[:, s_], in1=M2[:, s_, 0:EMB])
                vector.tensor_mul(out=prod[:, s_], in0=usel[:, s_], in1=msel[:, s_])
                vector.tensor_reduce(
                    out=res[:, s_], in_=prod[:, s_], axis=mybir.AxisListType.X, op=mybir.AluOpType.add
                ).then_inc(cmp_sem, 1)

    nc.compile()
    return nc, C, L16


def _wrap16(flat):
    n = flat.shape[0]
    blk = flat.reshape(n // 16, 16).T
    return np.tile(blk, (8, 1))


def build_core_inmap(c, plans, cnt_max, cap, L16, C, users, movies, user_table, movie_pairs):
    ncols = cap // 128
    colbase = {}
    cb = 0
    for i in range(N_UW):
        for j in range(N_MW):
            colbase[(i, j)] = cb
            cb += int(ncols[i, j])
    pl = plans[c]
    elems, uwl, mwl = pl["elems"], pl["uwl"], pl["mwl"]
    base = c * SHARD_ROWS
    end = min(N_USERS, base + SHARD_ROWS)
    shard = np.zeros((SHARD_ROWS, EMB), dtype=np.float32)
    shard[: end - base] = user_table[base:end]
    shard = shard.reshape(U_PAIRS, 2 * EMB)
    u_flat, m_flat, batch_ids, slots = [], [], [], []
    auflat = np.zeros(C * 128, dtype=np.float32)
    amflat = np.zeros(C * 128, dtype=np.float32)
    for i in range(N_UW):
        for j in range(N_MW):
            cell = elems[(uwl == i) & (mwl == j)]
            cnt = cell.shape[0]
            l16 = L16[i][j]
            ulocal = users[cell] - c * SHARD_ROWS
            ul = np.zeros(l16, dtype=np.int16)
            ul[:cnt] = ((ulocal >> 1) - i * WP).astype(np.int16)
            u_flat.append(ul)
            ml = np.zeros(l16, dtype=np.int16)
            ml[:cnt] = ((movies[cell] >> 1) - j * WP).astype(np.int16)
            m_flat.append(ml)
            s0 = colbase[(i, j)] * 128
            sl = s0 + np.arange(cnt)
            slots.append(sl)
            batch_ids.append(cell)
            auflat[sl] = (ulocal & 1).astype(np.float32)
            amflat[sl] = (movies[cell] & 1).astype(np.float32)
    return (
        {
            "user_shard": shard,
            "movie_table": movie_pairs,
            "u_idx": np.ascontiguousarray(_wrap16(np.concatenate(u_flat))),
            "m_idx": np.ascontiguousarray(_wrap16(np.concatenate(m_flat))),
            "au64": np.ascontiguousarray(np.broadcast_to(auflat.reshape(C, 128).T[:, :, None], (P, C, EMB)).astype(np.float32)),
            "am64": np.ascontiguousarray(np.broadcast_to(amflat.reshape(C, 128).T[:, :, None], (P, C, EMB)).astype(np.float32)),
        },
        np.concatenate(batch_ids),
        np.concatenate(slots),
    )


def _install_ntff_hook():
    """Shim antenv.axon_hooks (absent in this image) so trace=True works
    under axon, and disable the S3 artifact upload (zero-egress container)."""
    import sys
    import types

    import concourse.bass_utils as bu

    bu.upload_artifacts = lambda d: d

    try:
        from antenv.axon_hooks import get_axon_ntff_profile_hook  # noqa: F401

        return
    except ImportError:
        pass

    import antenv
    from trn_agent_boot.trn_boot import _ntff_profile_via_ctypes

    mod = types.ModuleType("antenv.axon_hooks")
    mod._hook = _ntff_profile_via_ctypes("/opt/axon/libaxon_pjrt.so")
    mod.set_axon_ntff_profile_hook = lambda h: setattr(mod, "_hook", h)
    mod.get_axon_ntff_profile_hook = lambda: mod._hook
    sys.modules["antenv.axon_hooks"] = mod
    antenv.axon_hooks = mod



def kernel(users, movies, user_table, movie_table):
    from concourse.bass_utils import run_bass_kernel_spmd

    users = np.asarray(users).astype(np.int64)
    movies = np.asarray(movies).astype(np.int64)
    user_table = np.ascontiguousarray(np.asarray(user_table, dtype=np.float32))
    movie_table = np.ascontiguousarray(np.asarray(movie_table, dtype=np.float32))
    movie_pairs = movie_table.reshape(M_PAIRS, 2 * EMB)

    plans, counts, cnt_max, cap = _plan(users, movies)
    key = (tuple(cap.ravel()), tuple(cnt_max.ravel()))
    if _CACHE.get("key") != key:
        _CACHE["nc"], _CACHE["C"], _CACHE["L16"] = _build_nc(cap, cnt_max)
        _CACHE["key"] = key
    nc, C, L16 = _CACHE["nc"], _CACHE["C"], _CACHE["L16"]

    in_maps = []
    slot_maps = []
    for c in range(N_CORES):
        in_map, bid, slt = build_core_inmap(
            c, plans, cnt_max, cap, L16, C, users, movies, user_table, movie_pairs
        )
        in_maps.append(in_map)
        slot_maps.append((bid, slt))

    trace = os.environ.get("KERNEL_TRACE", "") not in ("", "0")
    if trace:
        try:
            _install_ntff_hook()
        except Exception:
            trace = False
    res = run_bass_kernel_spmd(nc, in_maps, core_ids=list(range(N_CORES)), trace=trace)
    if trace:
        kernel.last_exec_time_ns = res.exec_time_ns
        kernel.last_trace = res.instructions_and_trace

    out = np.zeros((BATCH,), dtype=np.float32)
    for c in range(N_CORES):
        r = res.results[c]["out"]
        bid, slt = slot_maps[c]
        out[bid] = r[slt % 128, slt // 128]
    return out.reshape(BATCH, 1)
